# revision 37
# baseline (speedup 1.0000x reference)
"""Trainium2 Bass kernel for nn_AcPredict (banded basis-mixture Kalman predict).

Math (validated vs reference in numpy):
  All four basis stacks are band-masked (|i-j| <= 3), so the per-batch mixed
  transition matrices are 7-diagonal.  With D_m[b,i,t] = sum_k coeff[b,k] *
  basis_m[k,i,i+t-3]  (m in {11,12,21,22} -> 1..4) and S_x[b,i,t] = x[b,i+t-3]:

    nmu = mu + red_t(D1*S_mu + D2*S_ml)
    nml = ml + red_t(D3*S_mu + D4*S_ml)
    P1 = D1*S_cu + D2*S_cs ; P2 = D1*S_cs + D2*S_cl
    P3 = D3*S_cu + D4*S_cs ; P4 = D3*S_cs + D4*S_cl
    ncu = red_t(D1*P1 + D2*P2) + 2*P1[t=3] + cu + pcu
    ncl = red_t(D3*P3 + D4*P4) + 2*P4[t=3] + cl + pcl
    ncs = red_t(D3*P1 + D4*P2) + P2[t=3] + P3[t=3] + cs

Key structure choices:
  - Sharding: pure data-parallel, batch 4096 -> 8 cores x 512 rows (4 tiles
    of 128 partitions each).
  - Host prep does all layout work: pm pre-transposed (for the MLP),
    S-slot image pre-padded + pre-cast to bf16, weights pre-transposed and
    merged into one blob, process noise pre-broadcast into extra DMA rows.
  - The coeff MLP runs fully transposed ([feat, batch]); softmax is computed
    unnormalized and 1/sum(exp) is folded into the per-partition scale of
    the D-plane PSUM evacuation (sum(exp) comes from a 5th matmul reusing
    the same stationary as the D matmuls, so no transposes are needed).
  - The banded multiply pipeline (products, adds, covq pair-add) runs on DVE
    in bf16 (2x packed mode); the t-reductions (as 4-op add trees — gpsimd
    has no free-axis reduce), the +basepc/+pm adds and most output assembly
    run on the otherwise-idle Pool engine, leaving the DVE stream with no
    cross-engine waits in steady state.
  - Emission is software-pipelined: tile t+1's MLP is emitted right after
    tile t's DVE block so the reciprocal never trails the stream; tile t's
    assembly DMA is emitted one iteration later; the last tile runs its
    reduction chain on DVE (Pool would be the tail) and ships the means
    half of its output early.

Walrus caps sync-waits per compute instruction at 1: absorber warm-ups pin
DMA sems onto consuming engines' clocks; _split_multi_waits drains the rest.
"""

import sys

for _p in ("/opt/trn_rl_repo", "/opt/trn_rl_repo/concourse"):
    if _p not in sys.path:
        sys.path.insert(0, _p)

from contextlib import ExitStack

import ml_dtypes
import numpy as np

import concourse.bass as bass
import concourse.mybir as mybir
from concourse.bass import AP
from concourse.bass_utils import run_bass_kernel_spmd
from concourse.tile import TileContext

F32 = mybir.dt.float32
BF16 = mybir.dt.bfloat16
AX = mybir.AxisListType
OP = mybir.AluOpType
AF = mybir.ActivationFunctionType

B, LOD, LSD, LAD, K, BW, H = 4096, 64, 128, 32, 15, 3, 128
T = 2 * BW + 1          # 7 diagonals
NCORES = 8
R = B // NCORES         # rows per core = 512
P = 128                 # partitions per tile
NT = R // P             # tiles per core = 4
PL = LOD * T            # 448 elements per D plane
SL = LOD + 2 * BW       # 70 = padded slot width in x6 image

# pmtw blob columns: [w1t (128) | w2t (15) | ones15 | b1 | b2 | pmT (512)]
PW_W1 = 0
PW_W2 = PW_W1 + H       # 128
PW_ONE = PW_W2 + K      # 143
PW_B1 = PW_ONE + 1      # 144
PW_B2 = PW_B1 + 1       # 145
PW_PMT = PW_B2 + 1      # 146
PW_N = PW_PMT + NT * P  # 658


def _mk_ap(base, dims):
    """AP over `base` (an AP) with explicit extra free dims [[stride, n],...]."""
    return AP(tensor=base.tensor, offset=base.offset, ap=list(base.ap[:1]) + dims)


def _split_multi_waits(nc, cap=1):
    """Walrus caps sync-waits per instruction; spread extras over inserted
    drains on the same engine immediately before the offender."""
    for blk in nc.main_func.blocks:
        insts = blk.instructions
        i = 0
        while i < len(insts):
            inst = insts[i]
            si = getattr(inst, "sync_info", None)
            if si is not None and si.on_wait and len(si.on_wait) > cap:
                waits = list(si.on_wait)
                si.on_wait = waits[-cap:]
                extras = waits[:-cap]
                for j, w in enumerate(extras[::-1]):
                    d = mybir.InstDrain(
                        name=f"{inst.name}_wsplit{j}",
                        engine=inst.engine,
                        ins=[],
                        outs=[],
                        sync_info=mybir.SyncInfo(on_wait=[w], on_update=[]),
                    )
                    nc.register_instruction(d)
                    insts.insert(i, d)
                i += len(extras)
            i += 1


def build_bass():
    nc = bass.Bass()

    pm_d = nc.dram_tensor("pm", [R, LSD], F32, kind="ExternalInput")
    covx_d = nc.dram_tensor("covx", [R + P, 3 * LOD], F32, kind="ExternalInput")
    pmtw_d = nc.dram_tensor("pmtw", [P, PW_N], BF16, kind="ExternalInput")
    eb_d = nc.dram_tensor("eb", [K, 4 * PL], BF16, kind="ExternalInput")
    x6i_d = nc.dram_tensor("x6i", [R, 6 * SL], BF16, kind="ExternalInput")
    out_d = nc.dram_tensor("out", [R, 5 * LOD], F32, kind="ExternalOutput")

    with TileContext(nc) as tc, ExitStack() as ctx:
        const = ctx.enter_context(tc.tile_pool(name="const", bufs=1))
        ps = ctx.enter_context(tc.tile_pool(name="ps", bufs=1, space="PSUM"))

        def ctile(shape, dtype, tg):
            return const.tile(shape, dtype, tag=tg, name=tg)

        pm_sb = ctile([P, NT * LSD], F32, "pm_sb")
        cov_sb = ctile([P, 5 * 3 * LOD], F32, "cov_sb")
        pmtw_sb = ctile([P, PW_N], BF16, "pmtw_sb")
        eb_sb = ctile([P, 4 * PL], BF16, "eb_sb")
        x6_sb = ctile([P, NT * 6 * SL], BF16, "x6_sb")

        psD = ps.tile([P, 2048], F32, tag="psD", name="psD")
        psA = ps.tile([P, 512], F32, tag="psA", name="psA")
        psB = ps.tile([P, 512], F32, tag="psB", name="psB")

        h_bf = ctile([P, P], BF16, "h_bf")
        e_bf = ctile([P, P], BF16, "e_bf")
        rcp = ctile([P, 1], F32, "rcp")
        tmpAB = ctile([P, 6 * PL], BF16, "tmpAB")
        tmpCD = ctile([P, 6 * PL], BF16, "tmpCD")
        basepc = ctile([P, NT * 3 * LOD], F32, "basepc")

        rt3 = ctile([P, 576], BF16, "rt3")
        rt2 = ctile([P, 384], BF16, "rt2")
        d_bf = [ctile([P, 4 * PL], BF16, f"d_bf{i}") for i in range(2)]
        upp = [ctile([P, 6 * PL], BF16, f"upp{i}") for i in range(3)]
        vab = [ctile([P, 6 * PL], BF16, f"vab{i}") for i in range(2)]
        covq = [ctile([P, 3 * PL], BF16, f"covq{i}") for i in range(2)]
        ured = [ctile([P, 2 * LOD], F32, f"ured{i}") for i in range(2)]
        cqa = [ctile([P, 3 * LOD], F32, f"cqa{i}") for i in range(2)]
        tmc = [ctile([P, 3 * LOD], F32, f"tmc{i}") for i in range(2)]
        outb = [ctile([P, 5 * LOD], F32, f"outb{i}") for i in range(2)]
        absb = ctile([1, 8], BF16, "absb")
        absf = ctile([1, 8], F32, "absf")

        w1_sb = pmtw_sb[:, PW_W1 : PW_W1 + H]
        w2_sb = pmtw_sb[:, PW_W2 : PW_W2 + K]
        ones15 = pmtw_sb[0:K, PW_ONE : PW_ONE + 1]
        b1_sb = pmtw_sb[:, PW_B1 : PW_B1 + 1]
        b2_sb = pmtw_sb[0:K, PW_B2 : PW_B2 + 1]

        # ---- input DMAs ----
        # HWDGE on SP + ACT queues; big strided loads on gpsimd SWDGE (which
        # bypasses the shared HWDGE descriptor-gen device). pmtw gates the
        # whole pipeline (MLP weights + pmT) -> first on its own queue.
        # weights + pmT tile 0 first (gates the MLP); pmT tiles 1-3 after
        nc.sync.dma_start(
            pmtw_sb[:, 0 : PW_PMT + P], pmtw_d[:, 0 : PW_PMT + P]
        )
        nc.scalar.dma_start(
            pmtw_sb[:, PW_PMT + P :], pmtw_d[:, PW_PMT + P :]
        )
        nc.gpsimd.dma_start(eb_sb[0:K, :], eb_d[:, :])
        nc.gpsimd.dma_start(
            x6_sb[:].rearrange("p (t c) -> p t c", t=NT),
            _mk_ap(x6i_d[0:P, :], [[P * 6 * SL, NT], [1, 6 * SL]]),
        )
        nc.gpsimd.dma_start(
            cov_sb[:].rearrange("p (t c) -> p t c", t=5),
            _mk_ap(covx_d[0:P, :], [[P * 3 * LOD, 5], [1, 3 * LOD]]),
        )
        nc.gpsimd.dma_start(
            pm_sb[:].rearrange("p (t c) -> p t c", t=NT),
            _mk_ap(pm_d[0:P, :], [[P * LSD, NT], [1, LSD]]),
        )

        # ---- absorbers: pin DMA sems onto consuming engines' clocks ----
        nc.scalar.copy(absb[0:1, 0:1], pmtw_sb[0:1, 0:1])        # ACT <- pmtw
        nc.gpsimd.tensor_copy(absf[0:1, 0:1], pm_sb[0:1, 0:1])   # Pool <- pm

        # basepc[t] = cov[t] + pcb (pcb pre-broadcast into cov rows 512:640)
        nc.gpsimd.tensor_tensor(
            basepc[:].rearrange("p (t c) -> p t c", t=NT),
            cov_sb[:, 0 : NT * 3 * LOD].rearrange("p (t c) -> p t c", t=NT),
            _mk_ap(cov_sb[:, NT * 3 * LOD :], [[0, NT], [1, 3 * LOD]]),
            OP.add,
        )

        def _rep3(dm):
            """[128, 3, 448]: one D plane broadcast over 3 slots (0-stride)."""
            return _mk_ap(dm, [[0, 3], [1, PL]])

        def _sread(t, slot0):
            """[128, 3, 64, 7]: S[slot, i, t'] = x6[t][70*(slot0+s) + i + t']."""
            base = x6_sb[:, t * 6 * SL + slot0 * SL : t * 6 * SL + slot0 * SL + 1]
            return _mk_ap(base, [[SL, 3], [1, LOD], [1, T]])

        def emit_mlp(t):
            p = t % 2
            pmT = pmtw_sb[:, PW_PMT + t * P : PW_PMT + (t + 1) * P]
            nc.tensor.matmul(psA[:, 0:P], w1_sb, pmT)
            nc.scalar.activation(h_bf[:], psA[:, 0:P], AF.Tanh, bias=b1_sb)
            nc.tensor.matmul(psB[0:K, 0:P], w2_sb, h_bf[:])
            nc.scalar.activation(e_bf[0:K, :], psB[0:K, 0:P], AF.Exp, bias=b2_sb)
            nc.tensor.matmul(psB[:, P : P + 1], e_bf[0:K, :], ones15)
            nc.vector.reciprocal(rcp[:], psB[:, P : P + 1])
            for m in range(4):
                nc.tensor.matmul(
                    psD[:, 512 * m : 512 * m + PL],
                    e_bf[0:K, :],
                    eb_sb[0:K, PL * m : PL * (m + 1)],
                )
            for m in range(4):
                nc.scalar.mul(
                    d_bf[p][:, PL * m : PL * (m + 1)],
                    psD[:, 512 * m : 512 * m + PL],
                    rcp[:, 0:1],
                )

        def emit_dve_main(t):
            p = t % 2
            d = d_bf[p]
            # tmpAB = (D1*sA | D3*sA) ; tmpCD = (D2*sB | D4*sB)
            for br in range(2):
                nc.vector.tensor_tensor(
                    tmpAB[:, br * 3 * PL : (br + 1) * 3 * PL].rearrange(
                        "p (s x) -> p s x", s=3
                    ),
                    _rep3(d[:, 2 * br * PL : 2 * br * PL + PL]),
                    _sread(t, 0),
                    OP.mult,
                )
                nc.vector.tensor_tensor(
                    tmpCD[:, br * 3 * PL : (br + 1) * 3 * PL].rearrange(
                        "p (s x) -> p s x", s=3
                    ),
                    _rep3(d[:, (2 * br + 1) * PL : (2 * br + 2) * PL]),
                    _sread(t, 3),
                    OP.mult,
                )
            u = upp[t % 3]
            # upp = (U1,P1,P2 | U2,P3,P4)
            nc.vector.tensor_add(u[:], tmpAB[:], tmpCD[:])
            # vab = (D1P1, D2P2 | D3P3, D4P4 | D3P1, D4P2)
            nc.vector.tensor_tensor(
                vab[p][:, 0 : 4 * PL].rearrange("p (u x) -> p u x", u=2),
                d[:].rearrange("p (u x) -> p u x", u=2),
                _mk_ap(u[:, PL : PL + 1], [[3 * PL, 2], [1, 2 * PL]]),
                OP.mult,
            )
            nc.vector.tensor_mul(
                vab[p][:, 4 * PL : 6 * PL],
                d[:, 2 * PL : 4 * PL],
                u[:, PL : 3 * PL],
            )

        def _off(base, delta, dims):
            return AP(
                tensor=base.tensor,
                offset=base.offset + delta,
                ap=list(base.ap[:1]) + dims,
            )

        def _pool_tree(src_base, out_i, scratch, ncols):
            """out[i] = sum_t src[i*7 + t] for ncols i's, on Pool (gpsimd has
            no free-axis tensor_reduce): pairs (j, j+4) for j<3, then fold the
            3 partials and the t=3 leftover."""
            tt = nc.gpsimd.tensor_tensor
            sc = scratch[:, 0:1]
            tt(
                _mk_ap(sc, [[3, ncols], [1, 3]]),
                _off(src_base, 0, [[T, ncols], [1, 3]]),
                _off(src_base, 4, [[T, ncols], [1, 3]]),
                OP.add,
            )
            tt(out_i, _off(sc, 0, [[3, ncols]]), _off(sc, 1, [[3, ncols]]), OP.add)
            tt(out_i, out_i, _off(sc, 2, [[3, ncols]]), OP.add)
            tt(out_i, out_i, _off(src_base, 3, [[T, ncols]]), OP.add)

        def seg_reduce(eng, out_ui, src_base, nu, ustride, scratch):
            """out[u, i] = sum_t src[u, i, t], src elem (u,i,t) at
            src_base + u*ustride + i*T + t."""
            if eng is nc.vector:
                eng.reduce_sum(
                    out_ui.rearrange("p (u i) -> p u i", u=nu),
                    _mk_ap(src_base, [[ustride, nu], [T, LOD], [1, T]]),
                    axis=AX.X,
                )
                return
            assert ustride == LOD * T
            _pool_tree(src_base, out_ui, scratch, nu * LOD)

        def emit_cov_stage(t):
            p = t % 2
            u = upp[t % 3]
            # covq = (Q1+Q2, Q3+Q4, R1+R2) — DVE (2x bf16 beats Pool here)
            nc.vector.tensor_tensor(
                covq[p][:].rearrange("p (u x) -> p u x", u=3),
                _mk_ap(vab[p][:, 0:1], [[2 * PL, 3], [1, PL]]),
                _mk_ap(vab[p][:, PL : PL + 1], [[2 * PL, 3], [1, PL]]),
                OP.add,
            )
            # cqa = red_t(covq) + basepc; Pool for t<3, DVE tree for the tail
            eng = nc.vector if t == NT - 1 else nc.gpsimd
            if t == NT - 1:
                cq0 = covq[p][:, 0:1]
                sc = rt3[:, 0:1]
                nc.vector.tensor_tensor(
                    _mk_ap(sc, [[3, 3 * LOD], [1, 3]]),
                    _off(cq0, 0, [[T, 3 * LOD], [1, 3]]),
                    _off(cq0, 4, [[T, 3 * LOD], [1, 3]]),
                    OP.add,
                )
                nc.vector.tensor_tensor(
                    cqa[p][:],
                    _off(sc, 0, [[3, 3 * LOD]]),
                    _off(sc, 1, [[3, 3 * LOD]]),
                    OP.add,
                )
                nc.vector.tensor_tensor(
                    cqa[p][:], cqa[p][:], _off(sc, 2, [[3, 3 * LOD]]), OP.add
                )
                nc.vector.tensor_tensor(
                    cqa[p][:], cqa[p][:], _off(cq0, 3, [[T, 3 * LOD]]), OP.add
                )
            else:
                seg_reduce(eng, cqa[p][:], covq[p][:, 0:1], 3, PL, rt3)
            eng.tensor_tensor(
                cqa[p][:],
                cqa[p][:],
                basepc[:, t * 3 * LOD : (t + 1) * 3 * LOD],
                OP.add,
            )
            # ured: Pool tree per branch (strided chunks)
            for br in range(2):
                _pool_tree(
                    _off(u[:, 0:1], br * 3 * PL, []),
                    ured[p][:, br * LOD : (br + 1) * LOD],
                    rt2[:, br * 3 * LOD :],
                    LOD,
                )
            # outb[0:128] = ured + pm  (means)
            nc.gpsimd.tensor_tensor(
                outb[p][:, 0:LSD],
                ured[p][:],
                pm_sb[:, t * LSD : (t + 1) * LSD],
                OP.add,
            )
            if t == NT - 1:
                # tail: ship the means half early, covariances follow in
                # emit_asm — shortens the final DMA latency chain
                nc.sync.dma_start(
                    out_d[t * P : (t + 1) * P, 0:LSD], outb[p][:, 0:LSD]
                )
            # outb[256:320] (ncs) = (P2_3 + P3_3) + cqa_s
            teng = nc.vector if t == NT - 1 else nc.gpsimd
            teng.tensor_tensor(
                tmc[p][:, 0:LOD],
                _mk_ap(u[:, 2 * PL + 3 : 2 * PL + 4], [[T, LOD]]),
                _mk_ap(u[:, 4 * PL + 3 : 4 * PL + 4], [[T, LOD]]),
                OP.add,
            )
            nc.gpsimd.tensor_tensor(
                outb[p][:, 4 * LOD : 5 * LOD],
                tmc[p][:, 0:LOD],
                cqa[p][:, LSD : LSD + LOD],
                OP.add,
            )

        def emit_asm(t):
            p = t % 2
            u = upp[t % 3]
            # outb[128:256] (ncu|ncl) = 2*(P1_3, P4_3) + cqa[0:128], as two
            # Pool adds (P + (P + cqa)) — keeps DVE free of Pool-dependent ops
            pp = _mk_ap(u[:, PL + 3 : PL + 4], [[4 * PL, 2], [T, LOD]])
            cul = outb[p][:, LSD : 2 * LSD].rearrange("p (u i) -> p u i", u=2)
            cq = cqa[p][:, 0:LSD].rearrange("p (u i) -> p u i", u=2)
            if t == NT - 1:
                # tail: one DVE stt beats two serial Pool adds
                nc.vector.scalar_tensor_tensor(cul, pp, 2.0, cq, OP.mult, OP.add)
            else:
                nc.gpsimd.tensor_tensor(cul, pp, cq, OP.add)
                nc.gpsimd.tensor_tensor(cul, cul, pp, OP.add)
            if t == NT - 1:
                nc.sync.dma_start(
                    out_d[t * P : (t + 1) * P, LSD:], outb[p][:, LSD:]
                )
            else:
                nc.sync.dma_start(out_d[t * P : (t + 1) * P, :], outb[p][:])

        for t in range(NT):
            if t == 0:
                emit_mlp(0)
            emit_dve_main(t)
            if t + 1 < NT:
                emit_mlp(t + 1)
            if t > 0:
                emit_asm(t - 1)
            emit_cov_stage(t)
        emit_asm(NT - 1)

    _split_multi_waits(nc)
    return nc


_NC_CACHE = None


def _get_nc():
    global _NC_CACHE
    if _NC_CACHE is None:
        _NC_CACHE = build_bass()
    return _NC_CACHE


def _prep_shared(inputs):
    """Host prep shared across cores: E blob and the weight part of pmtw."""
    bsm = [inputs["basis11"], inputs["basis12"], inputs["basis21"], inputs["basis22"]]
    E = np.zeros((K, 4, LOD, T), np.float32)
    for m in range(4):
        for t in range(T):
            off = t - BW
            lo, hi = max(0, -off), min(LOD, LOD - off)
            E[:, m, lo:hi, t] = bsm[m][:, np.arange(lo, hi), np.arange(lo, hi) + off]
    eb = E.reshape(K, 4 * PL).astype(ml_dtypes.bfloat16)

    wtail = np.zeros((P, PW_PMT), ml_dtypes.bfloat16)
    wtail[:, PW_W1 : PW_W1 + H] = inputs["coeff_w1"].T.astype(ml_dtypes.bfloat16)
    wtail[:, PW_W2 : PW_W2 + K] = inputs["coeff_w2"].T.astype(ml_dtypes.bfloat16)
    wtail[0:K, PW_ONE] = ml_dtypes.bfloat16(1.0)
    wtail[:, PW_B1] = inputs["coeff_b1"].astype(ml_dtypes.bfloat16)
    wtail[0:K, PW_B2] = inputs["coeff_b2"].astype(ml_dtypes.bfloat16)

    lpn = inputs["log_process_noise"].astype(np.float32)
    pc = np.where(lpn < 0, np.exp(lpn), lpn + 1.0)[0]
    pcb_row = np.concatenate([pc[:LOD], pc[LOD:], np.zeros(LOD, np.float32)])
    return eb, wtail, pcb_row


def _prep_core(inputs, c, eb, wtail, pcb_row):
    sl = slice(c * R, (c + 1) * R)
    pm = np.ascontiguousarray(inputs["post_mean"][sl]).astype(np.float32)
    cu = inputs["post_cov_u"][sl].astype(np.float32)
    clo = inputs["post_cov_l"][sl].astype(np.float32)
    cs = inputs["post_cov_s"][sl].astype(np.float32)

    covx = np.empty((R + P, 3 * LOD), np.float32)
    covx[:R, 0:LOD] = cu
    covx[:R, LOD : 2 * LOD] = clo
    covx[:R, 2 * LOD :] = cs
    covx[R:] = pcb_row

    pmtw = np.empty((P, PW_N), ml_dtypes.bfloat16)
    pmtw[:, 0:PW_PMT] = wtail
    pmtw[:, PW_PMT:] = pm.T.astype(ml_dtypes.bfloat16)

    pmb = pm.astype(ml_dtypes.bfloat16)
    x6i = np.zeros((R, 6 * SL), ml_dtypes.bfloat16)
    slot_src = [
        pmb[:, 0:LOD],
        cu.astype(ml_dtypes.bfloat16),
        cs.astype(ml_dtypes.bfloat16),
        pmb[:, LOD:],
        cs.astype(ml_dtypes.bfloat16),
        clo.astype(ml_dtypes.bfloat16),
    ]
    for s, src in enumerate(slot_src):
        x6i[:, s * SL + BW : s * SL + BW + LOD] = src

    return dict(pm=pm, covx=covx, pmtw=pmtw, eb=eb, x6i=x6i)


def kernel(**inputs):
    return _run(inputs, trace=False)[0]


def _run(inputs, trace=False, tmpdir=None):
    inputs = {k: np.asarray(v) for k, v in inputs.items()}
    eb, wtail, pcb_row = _prep_shared(inputs)
    nc = _get_nc()

    in_maps = [_prep_core(inputs, c, eb, wtail, pcb_row) for c in range(NCORES)]
    res = run_bass_kernel_spmd(
        nc, in_maps, list(range(NCORES)), trace=trace, tmpdir=tmpdir
    )
    outs = [np.asarray(res.results[c]["out"]) for c in range(NCORES)]
    return np.concatenate(outs, axis=0).astype(np.float32), res
